# revision 17
# baseline (speedup 1.0000x reference)
"""Causal multi-head attention on 8 Trainium2 NeuronCores.

Problem (hardcoded): B=4, T=2048, C=1024, H=16, hd=64.
  qkv = x @ Wqkv + bqkv ; causal softmax attention ; out = attn_out @ Wout + bout
  returns (out, k, v) with k, v in [B, H, T, hd].

Sharding: 8 cores = 4 batches x 2 head-groups (8 heads each).
Core c handles batch b=c//2, head group g=c%2; the host sums the two
output-projection partials per batch.

Device layouts (per core):
  xT   [C=1024, T=2048]  (host-transposed x[b])
  qT/kT [j=512, t=2048]  bf16; tile hp packs head pair (2hp, 2hp+1):
                         partitions 0-63 head 2hp, 64-127 head 2hp+1
  v_sb  16 tiles [t=128, 8*65] bf16: per head 64 v-dims + a ones column
        (the ones column makes the attention matmul emit the softmax
        denominator as output row 64)
  ST = scores^T [tk, tq] in PSUM; exp on ScalarE (scale=1/8 folded in);
  denominator reciprocal via DVE reciprocal_approx_fast;
  broadcast across partitions via GpSimd partition_broadcast.
  outT [j=512, t=2048] -> final projection (PROJ_DT) -> y [t, ycols].
"""

import os
import numpy as np
import ml_dtypes

B, T, C, H = 4, 2048, 1024, 16
HD = C // H            # 64
NCORES = 8
HPC = H // 2           # heads per core = 8
JC = HPC * HD          # per-core head width = 512
NT = T // 128          # 16 t-blocks
NS = T // 512          # 4 t-stripes
NCT = C // 128         # 8 c-tiles
NJT = HPC              # 8 local heads

PROJ_DT = "f16"        # "f16" | "f32r" | "f32" for projection matmuls


def _build_program(loop_k: int = 1):
    import concourse.bass as bass
    import concourse.bacc as bacc
    import concourse.tile as tile
    import concourse.mybir as mybir

    f32 = mybir.dt.float32
    f32r = mybir.dt.float32r
    bf16 = mybir.dt.bfloat16
    AF = mybir.ActivationFunctionType
    fmm = {"f16": mybir.dt.float16, "f32r": f32r, "f32": f32}[PROJ_DT]

    nc = bacc.Bacc("TRN2", target_bir_lowering=False, debug=False)

    xT = nc.dram_tensor("xT", [C, T], fmm, kind="ExternalInput").ap()
    Wqk = nc.dram_tensor("Wqk", [C, 2 * JC], fmm, kind="ExternalInput").ap()
    Wv = nc.dram_tensor("Wv", [C, JC], fmm, kind="ExternalInput").ap()
    Wo = nc.dram_tensor("Wo", [JC, C], fmm, kind="ExternalInput").ap()
    bqk = nc.dram_tensor("bqk", [2 * JC], f32, kind="ExternalInput").ap()
    bv = nc.dram_tensor("bv", [JC], f32, kind="ExternalInput").ap()
    tri = nc.dram_tensor("tri", [128, 128], bf16, kind="ExternalInput").ap()

    kT_out = nc.dram_tensor("kT_out", [JC, T], f32, kind="ExternalOutput").ap()
    v_out = nc.dram_tensor("v_out", [T, JC], f32, kind="ExternalOutput").ap()
    y_out = nc.dram_tensor("y_out", [T, C], f32, kind="ExternalOutput").ap()


    with tile.TileContext(nc) as tc:
        with (
            tc.tile_pool(name="persist", bufs=1) as persist,
            tc.tile_pool(name="wqk", bufs=2) as wqk_pool,
            tc.tile_pool(name="xs", bufs=2) as xs_pool,
            tc.tile_pool(name="pp", bufs=2, space="PSUM") as proj_psum,
            tc.tile_pool(name="stp", bufs=3, space="PSUM") as st_psum,
            tc.tile_pool(name="op", bufs=3, space="PSUM") as o_psum,
            tc.tile_pool(name="stage", bufs=3) as stage_pool,
            tc.tile_pool(name="pt", bufs=8) as p_pool,
            tc.tile_pool(name="qs", bufs=3) as qs_pool,
            tc.tile_pool(name="small", bufs=3) as small_pool,
            tc.tile_pool(name="bc", bufs=3) as bc_pool,
        ):
            # ---- persistent tiles ----
            kT = [persist.tile([128, T], bf16, name=f"kT{i}", tag=f"kT{i}")
                  for i in range(4)]
            v_sb = [persist.tile([128, NJT * (HD + 1)], bf16, name=f"vsb{i}",
                                 tag=f"vsb{i}") for i in range(NT)]
            outT = [persist.tile([128, T], fmm, name=f"outT{i}", tag=f"outT{i}")
                    for i in range(4)]
            wv_sb = [persist.tile([128, JC], fmm, name=f"wv{i}", tag=f"wv{i}")
                     for i in range(NCT)]
            wo_sb = [persist.tile([128, C], fmm, name=f"wo{i}", tag=f"wo{i}")
                     for i in range(4)]
            tri_sb = persist.tile([128, 128], bf16, name="tri_sb", tag="tri_sb")
            bqk_sb = persist.tile([128, 8], f32, name="bqk_sb", tag="bqk_sb")
            bv_sb = persist.tile([1, JC], f32, name="bv_sb", tag="bv_sb")
            vb_bc = persist.tile([128, JC], f32, name="vb_bc", tag="vb_bc")

            nc.sync.dma_start(tri_sb[:], tri[:])
            nc.sync.dma_start(bqk_sb[:], bqk.rearrange("(j p) -> p j", p=128))
            nc.sync.dma_start(bv_sb[:], bv.rearrange("(p j) -> p j", p=1))
            nc.gpsimd.partition_broadcast(vb_bc[:], bv_sb[:])

            # ones columns of v_sb (col 64 of each 65-wide head slot)
            for tt in range(NT):
                for lh in range(NJT):
                    c = lh * (HD + 1) + HD
                    nc.vector.memset(v_sb[tt][:, c:c + 1], 1.0)

            # ---- phase A: projections, streamed over t-stripes of 512 ----
            def proj_stripe(s):
                t0 = s * 512
                xs = [xs_pool.tile([128, 512], fmm, name=f"xs{ct}",
                                   tag=f"xs{ct}") for ct in range(NCT)]
                for ct in range(NCT):
                    nc.sync.dma_start(xs[ct][:],
                                      xT[ct * 128:(ct + 1) * 128, t0:t0 + 512])
                wblk = [wqk_pool.tile([128, 2 * JC], fmm, name=f"wqk{ct}",
                                      tag=f"wqk{ct}") for ct in range(NCT)]
                for ct in range(NCT):
                    nc.sync.dma_start(wblk[ct][:],
                                      Wqk[ct * 128:(ct + 1) * 128, :])
                qs = [qs_pool.tile([128, 512], bf16, name=f"qs{i}",
                                   tag=f"qs{i}") for i in range(4)]
                # q/k projection: out [j=128, t=512], W stationary
                for jt in range(8):
                    ps = proj_psum.tile([128, 512], f32, name="ps", tag="ps")
                    for ct in range(NCT):
                        nc.tensor.matmul(ps[:],
                                         wblk[ct][:, jt * 128:(jt + 1) * 128],
                                         xs[ct][:],
                                         start=(ct == 0), stop=(ct == NCT - 1))
                    bias = bqk_sb[:, jt:jt + 1]
                    if jt < 4:  # q part
                        nc.vector.tensor_scalar_add(qs[jt][:], ps[:], bias)
                    else:       # k part: exact f32 to HBM, bf16 for QK
                        kst = stage_pool.tile([128, 512], f32, name="kst",
                                              tag="stage")
                        nc.vector.tensor_scalar_add(kst[:], ps[:], bias)
                        nc.vector.tensor_copy(
                            kT[jt - 4][:, t0:t0 + 512], kst[:])
                        nc.sync.dma_start(
                            kT_out[(jt - 4) * 128:(jt - 3) * 128, t0:t0 + 512],
                            kst[:])
                # v projection: out [t=128, j=512], xT stationary
                if s == 0:
                    for i in range(NCT):
                        nc.sync.dma_start(wv_sb[i][:],
                                          Wv[i * 128:(i + 1) * 128, :])
                for tt4 in range(4):
                    tt = s * 4 + tt4
                    ps = proj_psum.tile([128, 512], f32, name="ps", tag="ps")
                    for ct in range(NCT):
                        nc.tensor.matmul(
                            ps[:],
                            xs[ct][:, tt4 * 128:(tt4 + 1) * 128],
                            wv_sb[ct][:],
                            start=(ct == 0), stop=(ct == NCT - 1))
                    vst = stage_pool.tile([128, 512], f32, name="vst",
                                          tag="stage")
                    nc.vector.tensor_add(vst[:], ps[:], vb_bc[:])
                    nc.sync.dma_start(v_out[tt * 128:(tt + 1) * 128, :], vst[:])
                    for lh in range(NJT):
                        nc.vector.tensor_copy(
                            v_sb[tt][:, lh * (HD + 1):lh * (HD + 1) + HD],
                            vst[:, lh * HD:(lh + 1) * HD])
                return qs

            # ---- phase B: attention for head pair hp, tq-stripe s ----
            def attn(hp, s, qs):
                t0 = s * 512
                oA = o_psum.tile([65, 512], f32, name="oA", tag="o")
                oB = o_psum.tile([65, 512], f32, name="oB", tag="o")
                vlo = 2 * hp * (HD + 1)
                vhi = (2 * hp + 1) * (HD + 1)
                n_i = 4 * s + 4
                scale = float(HD) ** -0.5
                for i in range(n_i):
                    r = i - 4 * s  # >=0 only on diagonal-boundary blocks
                    c0 = max(0, r) * 128  # first valid tq col in stripe
                    stA = st_psum.tile([128, 512], f32, name="stA", tag="st")
                    stB = st_psum.tile([128, 512], f32, name="stB", tag="st")
                    nc.tensor.matmul(stA[:, c0:512],
                                     kT[hp][0:64, i * 128:(i + 1) * 128],
                                     qs[hp][0:64, c0:512],
                                     start=True, stop=True)
                    nc.tensor.matmul(stB[:, c0:512],
                                     kT[hp][64:128, i * 128:(i + 1) * 128],
                                     qs[hp][64:128, c0:512],
                                     start=True, stop=True)
                    pA = p_pool.tile([128, 512], bf16, name="pA", tag="p")
                    pB = p_pool.tile([128, 512], bf16, name="pB", tag="p")
                    nc.scalar.activation(pA[:, c0:512], stA[:, c0:512],
                                         AF.Exp, scale=scale)
                    nc.scalar.activation(pB[:, c0:512], stB[:, c0:512],
                                         AF.Exp, scale=scale)
                    if r >= 0:  # mask the diagonal 128x128 sub-block
                        nc.vector.tensor_mul(pA[:, c0:c0 + 128],
                                             pA[:, c0:c0 + 128], tri_sb[:])
                        nc.vector.tensor_mul(pB[:, c0:c0 + 128],
                                             pB[:, c0:c0 + 128], tri_sb[:])
                    nc.tensor.matmul(oA[:, c0:512], v_sb[i][:, vlo:vlo + 65],
                                     pA[:, c0:512],
                                     start=(i == 0), stop=(i == n_i - 1))
                    nc.tensor.matmul(oB[:, c0:512], v_sb[i][:, vhi:vhi + 65],
                                     pB[:, c0:512],
                                     start=(i == 0), stop=(i == n_i - 1))
                # normalize: 1/denom = exp(-ln(d)); broadcast over partitions
                for half, o in ((0, oA), (1, oB)):
                    d64 = small_pool.tile([65, 512], f32, name="d64",
                                          tag="d64")
                    nc.vector.tensor_copy(d64[64:65, :], o[64:65, :])
                    d0 = small_pool.tile([1, 512], f32, name="d0", tag="d0")
                    nc.sync.dma_start(d0[:], d64[64:65, :])
                    rcp = small_pool.tile([1, 512], f32, name="rcp", tag="rcp")
                    nc.vector.reciprocal_approx_fast(rcp[:], d0[:])
                    bc = bc_pool.tile([64, 512], f32, name="bc", tag="bc")
                    nc.gpsimd.partition_broadcast(bc[:], rcp[:])
                    if half == 0:
                        nc.vector.tensor_mul(outT[hp][0:64, t0:t0 + 512],
                                             o[0:64, :], bc[:])
                    else:
                        nB = stage_pool.tile([64, 512], fmm, name="nB",
                                             tag="nstage")
                        nc.vector.tensor_mul(nB[:], o[0:64, :], bc[:])
                        nc.sync.dma_start(outT[hp][64:128, t0:t0 + 512],
                                          nB[:])

            # ---- phase C: output projection [t=128, ycols=512] ----
            def outproj(tt):
                for ys in range(2):
                    ps = proj_psum.tile([128, 512], f32, name="ps", tag="ps")
                    for jt in range(4):
                        nc.tensor.matmul(
                            ps[:],
                            outT[jt][:, tt * 128:(tt + 1) * 128],
                            wo_sb[jt][:, ys * 512:(ys + 1) * 512],
                            start=(jt == 0), stop=(jt == 3))
                    yst = stage_pool.tile([128, 512], f32, name="yst",
                                          tag="stage")
                    nc.vector.tensor_copy(yst[:], ps[:])
                    nc.sync.dma_start(
                        y_out[tt * 128:(tt + 1) * 128, ys * 512:(ys + 1) * 512],
                        yst[:])

            # emission order: pipeline proj stripes with attention waves
            def emit_body():
                qs0 = proj_stripe(0)
                qs1 = proj_stripe(1)
                for hp in range(4):
                    attn(hp, 0, qs0)
                for i in range(4):
                    nc.sync.dma_start(wo_sb[i][:],
                                      Wo[i * 128:(i + 1) * 128, :])
                qs2 = proj_stripe(2)
                for hp in range(4):
                    attn(hp, 1, qs1)
                for tt in range(4):
                    outproj(tt)
                qs3 = proj_stripe(3)
                for hp in range(4):
                    attn(hp, 2, qs2)
                for tt in range(4, 8):
                    outproj(tt)
                for hp in range(4):
                    attn(hp, 3, qs3)
                for tt in range(8, NT):
                    outproj(tt)

            if loop_k == 1:
                emit_body()
            else:
                with tc.For_i(0, loop_k, 1):
                    emit_body()

    nc.compile()
    return nc


_NC_CACHE = {}


def _get_program():
    if "nc" not in _NC_CACHE:
        _NC_CACHE["nc"] = _build_program()
    return _NC_CACHE["nc"]


def _make_tri():
    # valid (tk <= tq) in ST layout [tk, tq] = upper triangular incl diagonal
    p = np.arange(128)[:, None]
    f = np.arange(128)[None, :]
    return (p <= f).astype(ml_dtypes.bfloat16)


def make_in_maps(x, Wqkv, bqkv, Wout):
    tri = _make_tri()
    in_maps = []
    for c in range(NCORES):
        b, g = divmod(c, 2)
        j0 = g * JC
        Wq = Wqkv[:, j0:j0 + JC]
        Wk = Wqkv[:, C + j0:C + j0 + JC]
        Wv_ = Wqkv[:, 2 * C + j0:2 * C + j0 + JC]
        mmdt = np.float16 if PROJ_DT == "f16" else np.float32
        in_maps.append({
            "xT": np.ascontiguousarray(x[b].T).astype(mmdt),
            "Wqk": np.ascontiguousarray(
                np.concatenate([Wq, Wk], axis=1)).astype(mmdt),
            "Wv": np.ascontiguousarray(Wv_).astype(mmdt),
            "Wo": np.ascontiguousarray(Wout[j0:j0 + JC, :]).astype(mmdt),
            "bqk": np.ascontiguousarray(
                np.concatenate([bqkv[j0:j0 + JC], bqkv[C + j0:C + j0 + JC]])),
            "bv": np.ascontiguousarray(bqkv[2 * C + j0:2 * C + j0 + JC]),
            "tri": tri,
        })
    return in_maps


def kernel(x, Wqkv, bqkv, Wout, bout):
    from concourse.bass_utils import run_bass_kernel_spmd

    x = np.asarray(x, dtype=np.float32)
    Wqkv = np.asarray(Wqkv, dtype=np.float32)
    bqkv = np.asarray(bqkv, dtype=np.float32)
    Wout = np.asarray(Wout, dtype=np.float32)
    bout = np.asarray(bout, dtype=np.float32)

    in_maps = make_in_maps(x, Wqkv, bqkv, Wout)
    nc = _get_program()
    trace = bool(int(os.environ.get("KERNEL_TRACE", "0")))
    res = run_bass_kernel_spmd(nc, in_maps, list(range(NCORES)), trace=trace)
    results = res.results
    _NC_CACHE["last_exec_time_ns"] = res.exec_time_ns

    out = np.zeros((B, T, C), dtype=np.float32)
    k = np.zeros((B, H, T, HD), dtype=np.float32)
    v = np.zeros((B, H, T, HD), dtype=np.float32)
    for c in range(NCORES):
        b, g = divmod(c, 2)
        r = results[c]
        out[b] += r["y_out"]
        for lh in range(HPC):
            h = g * HPC + lh
            k[b, h] = r["kT_out"][lh * HD:(lh + 1) * HD, :].T
            v[b, h] = r["v_out"][:, lh * HD:(lh + 1) * HD]
    out += bout[None, None, :]
    return out, k, v


# revision 21
# speedup vs baseline: 1.8443x; 1.8443x over previous
"""Causal multi-head attention on 8 Trainium2 NeuronCores.

Problem (hardcoded): B=4, T=2048, C=1024, H=16, hd=64.
  qkv = x @ Wqkv + bqkv ; causal softmax attention ; out = attn_out @ Wout + bout
  returns (out, k, v) with k, v in [B, H, T, hd].

Sharding: 8 cores = 4 batches x 2 head-groups (8 heads each).
Core c handles batch b=c//2, head group g=c%2; the host sums the two
output-projection partials per batch.

Device layouts (per core):
  xT   [C=1024, T=2048]  (host-transposed x[b])
  qT/kT [j=512, t=2048]  bf16; tile hp packs head pair (2hp, 2hp+1):
                         partitions 0-63 head 2hp, 64-127 head 2hp+1
  v_sb  16 tiles [t=128, 8*65] bf16: per head 64 v-dims + a ones column
        (the ones column makes the attention matmul emit the softmax
        denominator as output row 64)
  ST = scores^T [tk, tq] in PSUM; exp on ScalarE (scale=1/8 folded in);
  denominator reciprocal via DVE reciprocal_approx_fast;
  broadcast across partitions via GpSimd partition_broadcast.
  outT [j=512, t=2048] -> final projection (PROJ_DT) -> y [t, ycols].
"""

import os
import numpy as np
import ml_dtypes

B, T, C, H = 4, 2048, 1024, 16
HD = C // H            # 64
NCORES = 8
HPC = H // 2           # heads per core = 8
JC = HPC * HD          # per-core head width = 512
NT = T // 128          # 16 t-blocks
NS = T // 512          # 4 t-stripes
NCT = C // 128         # 8 c-tiles
NJT = HPC              # 8 local heads

PROJ_DT = "f16"        # "f16" | "f32r" | "f32" for projection matmuls


def _build_program(loop_k: int = 1):
    import concourse.bass as bass
    import concourse.bacc as bacc
    import concourse.tile as tile
    import concourse.mybir as mybir

    f32 = mybir.dt.float32
    f32r = mybir.dt.float32r
    bf16 = mybir.dt.bfloat16
    AF = mybir.ActivationFunctionType
    fmm = {"f16": mybir.dt.float16, "f32r": f32r, "f32": f32}[PROJ_DT]

    nc = bacc.Bacc("TRN2", target_bir_lowering=False, debug=False)

    xT = nc.dram_tensor("xT", [C, T], fmm, kind="ExternalInput").ap()
    Wqk = nc.dram_tensor("Wqk", [C, 2 * JC], fmm, kind="ExternalInput").ap()
    Wv = nc.dram_tensor("Wv", [C, JC], fmm, kind="ExternalInput").ap()
    Wo = nc.dram_tensor("Wo", [JC, C], fmm, kind="ExternalInput").ap()
    bqk = nc.dram_tensor("bqk", [2 * JC], f32, kind="ExternalInput").ap()
    bv = nc.dram_tensor("bv", [JC], f32, kind="ExternalInput").ap()
    tri = nc.dram_tensor("tri", [128, 128], bf16, kind="ExternalInput").ap()

    kT_out = nc.dram_tensor("kT_out", [JC, T], f32, kind="ExternalOutput").ap()
    v_out = nc.dram_tensor("v_out", [T, JC], f32, kind="ExternalOutput").ap()
    y_out = nc.dram_tensor("y_out", [T, C], f32, kind="ExternalOutput").ap()


    with tile.TileContext(nc) as tc:
        with (
            tc.tile_pool(name="persist", bufs=1) as persist,
            tc.tile_pool(name="wqk", bufs=2) as wqk_pool,
            tc.tile_pool(name="xs", bufs=2) as xs_pool,
            tc.tile_pool(name="pp", bufs=2, space="PSUM") as proj_psum,
            tc.tile_pool(name="stp", bufs=2, space="PSUM") as st_psum,
            tc.tile_pool(name="op", bufs=2, space="PSUM") as o_psum,
            tc.tile_pool(name="stage", bufs=3) as stage_pool,
            tc.tile_pool(name="pt", bufs=6) as p_pool,
            tc.tile_pool(name="qs", bufs=3) as qs_pool,
            tc.tile_pool(name="small", bufs=3) as small_pool,
            tc.tile_pool(name="bc", bufs=3) as bc_pool,
        ):
            # ---- persistent tiles ----
            kT = [persist.tile([128, T], bf16, name=f"kT{i}", tag=f"kT{i}")
                  for i in range(4)]
            v_sb = [persist.tile([128, NJT * (HD + 1)], bf16, name=f"vsb{i}",
                                 tag=f"vsb{i}") for i in range(NT)]
            outT = [persist.tile([128, T], fmm, name=f"outT{i}", tag=f"outT{i}")
                    for i in range(4)]
            wv_sb = [persist.tile([128, JC], fmm, name=f"wv{i}", tag=f"wv{i}")
                     for i in range(NCT)]
            wo_sb = [persist.tile([128, C], fmm, name=f"wo{i}", tag=f"wo{i}")
                     for i in range(4)]
            tri_sb = persist.tile([128, 128], bf16, name="tri_sb", tag="tri_sb")
            bqk_sb = persist.tile([128, 8], f32, name="bqk_sb", tag="bqk_sb")
            bv_sb = persist.tile([1, JC], f32, name="bv_sb", tag="bv_sb")
            vb_bc = persist.tile([128, JC], f32, name="vb_bc", tag="vb_bc")

            nc.sync.dma_start(tri_sb[:], tri[:])
            nc.sync.dma_start(bqk_sb[:], bqk.rearrange("(j p) -> p j", p=128))
            nc.sync.dma_start(bv_sb[:], bv.rearrange("(p j) -> p j", p=1))
            nc.gpsimd.partition_broadcast(vb_bc[:], bv_sb[:])

            # ones columns of v_sb (col 64 of each 65-wide head slot)
            for tt in range(NT):
                for lh in range(NJT):
                    c = lh * (HD + 1) + HD
                    nc.vector.memset(v_sb[tt][:, c:c + 1], 1.0)

            # ---- phase A: projections, streamed over t-stripes of 512 ----
            def proj_stripe(s):
                t0 = s * 512
                xs = [xs_pool.tile([128, 512], fmm, name=f"xs{ct}",
                                   tag=f"xs{ct}") for ct in range(NCT)]
                for ct in range(NCT):
                    nc.sync.dma_start(xs[ct][:],
                                      xT[ct * 128:(ct + 1) * 128, t0:t0 + 512])
                wblk = [wqk_pool.tile([128, 2 * JC], fmm, name=f"wqk{ct}",
                                      tag=f"wqk{ct}") for ct in range(NCT)]
                for ct in range(NCT):
                    nc.sync.dma_start(wblk[ct][:],
                                      Wqk[ct * 128:(ct + 1) * 128, :])
                qs = [qs_pool.tile([128, 512], bf16, name=f"qs{i}",
                                   tag=f"qs{i}") for i in range(4)]
                # q/k projection: out [j=128, t=512], W stationary
                for jt in range(8):
                    ps = proj_psum.tile([128, 512], f32, name="ps", tag="ps")
                    for ct in range(NCT):
                        nc.tensor.matmul(ps[:],
                                         wblk[ct][:, jt * 128:(jt + 1) * 128],
                                         xs[ct][:],
                                         start=(ct == 0), stop=(ct == NCT - 1))
                    bias = bqk_sb[:, jt:jt + 1]
                    if jt < 4:  # q part
                        nc.vector.tensor_scalar_add(qs[jt][:], ps[:], bias)
                    else:       # k part: exact f32 to HBM, bf16 for QK
                        kst = stage_pool.tile([128, 512], f32, name="kst",
                                              tag="stage")
                        nc.vector.tensor_scalar_add(kst[:], ps[:], bias)
                        nc.vector.tensor_copy(
                            kT[jt - 4][:, t0:t0 + 512], kst[:])
                        nc.sync.dma_start(
                            kT_out[(jt - 4) * 128:(jt - 3) * 128, t0:t0 + 512],
                            kst[:])
                # v projection: out [t=128, j=512], xT stationary
                if s == 0:
                    for i in range(NCT):
                        nc.sync.dma_start(wv_sb[i][:],
                                          Wv[i * 128:(i + 1) * 128, :])
                for tt4 in range(4):
                    tt = s * 4 + tt4
                    ps = proj_psum.tile([128, 512], f32, name="ps", tag="ps")
                    for ct in range(NCT):
                        nc.tensor.matmul(
                            ps[:],
                            xs[ct][:, tt4 * 128:(tt4 + 1) * 128],
                            wv_sb[ct][:],
                            start=(ct == 0), stop=(ct == NCT - 1))
                    vst = stage_pool.tile([128, 512], f32, name="vst",
                                          tag="stage")
                    nc.vector.tensor_add(vst[:], ps[:], vb_bc[:])
                    nc.sync.dma_start(v_out[tt * 128:(tt + 1) * 128, :], vst[:])
                    for lh in range(NJT):
                        nc.vector.tensor_copy(
                            v_sb[tt][:, lh * (HD + 1):lh * (HD + 1) + HD],
                            vst[:, lh * HD:(lh + 1) * HD])
                return qs

            # ---- phase B: attention for head pair hp, tq-stripe s ----
            def attn(hp, s, qs):
                t0 = s * 512
                oA = o_psum.tile([65, 512], f32, name="oA", tag="o")
                oB = o_psum.tile([65, 512], f32, name="oB", tag="o")
                vlo = 2 * hp * (HD + 1)
                vhi = (2 * hp + 1) * (HD + 1)
                n_i = 4 * s + 4
                scale = float(HD) ** -0.5
                for i in range(n_i):
                    r = i - 4 * s  # >=0 only on diagonal-boundary blocks
                    c0 = max(0, r) * 128  # first valid tq col in stripe
                    stAB = st_psum.tile([128, 1024], f32, name="stAB",
                                        tag="st")
                    nc.tensor.matmul(stAB[:, c0:512],
                                     kT[hp][0:64, i * 128:(i + 1) * 128],
                                     qs[hp][0:64, c0:512],
                                     start=True, stop=True)
                    nc.tensor.matmul(stAB[:, 512 + c0:1024],
                                     kT[hp][64:128, i * 128:(i + 1) * 128],
                                     qs[hp][64:128, c0:512],
                                     start=True, stop=True)
                    pAB = p_pool.tile([128, 1024], bf16, name="pAB", tag="p")
                    # one exp over both heads' halves (same causal geometry)
                    st3 = stAB.rearrange("p (h w) -> p h w", h=2)[:, :, c0:512]
                    pp3 = pAB.rearrange("p (h w) -> p h w", h=2)[:, :, c0:512]
                    nc.scalar.activation(pp3, st3, AF.Exp, scale=scale)
                    if r >= 0:  # mask the diagonal 128x128 sub-block
                        nc.vector.tensor_mul(pAB[:, c0:c0 + 128],
                                             pAB[:, c0:c0 + 128], tri_sb[:])
                        nc.vector.tensor_mul(
                            pAB[:, 512 + c0:512 + c0 + 128],
                            pAB[:, 512 + c0:512 + c0 + 128], tri_sb[:])
                    nc.tensor.matmul(oA[:, c0:512], v_sb[i][:, vlo:vlo + 65],
                                     pAB[:, c0:512],
                                     start=(i == 0), stop=(i == n_i - 1))
                    nc.tensor.matmul(oB[:, c0:512], v_sb[i][:, vhi:vhi + 65],
                                     pAB[:, 512 + c0:1024],
                                     start=(i == 0), stop=(i == n_i - 1))
                # normalize: 1/denom = exp(-ln(d)); broadcast over partitions
                for half, o in ((0, oA), (1, oB)):
                    d64 = small_pool.tile([65, 512], f32, name="d64",
                                          tag="d64")
                    nc.vector.tensor_copy(d64[64:65, :], o[64:65, :])
                    d0 = small_pool.tile([1, 512], f32, name="d0", tag="d0")
                    nc.sync.dma_start(d0[:], d64[64:65, :])
                    rcp = small_pool.tile([1, 512], f32, name="rcp", tag="rcp")
                    nc.vector.reciprocal_approx_fast(rcp[:], d0[:])
                    bc = bc_pool.tile([64, 512], f32, name="bc", tag="bc")
                    nc.gpsimd.partition_broadcast(bc[:], rcp[:])
                    if half == 0:
                        nc.vector.tensor_mul(outT[hp][0:64, t0:t0 + 512],
                                             o[0:64, :], bc[:])
                    else:
                        nB = stage_pool.tile([64, 512], fmm, name="nB",
                                             tag="nstage")
                        nc.vector.tensor_mul(nB[:], o[0:64, :], bc[:])
                        nc.sync.dma_start(outT[hp][64:128, t0:t0 + 512],
                                          nB[:])

            # ---- phase C: output projection [t=128, ycols=512] ----
            def outproj(tt):
                for ys in range(2):
                    ps = proj_psum.tile([128, 512], f32, name="ps", tag="ps")
                    for jt in range(4):
                        nc.tensor.matmul(
                            ps[:],
                            outT[jt][:, tt * 128:(tt + 1) * 128],
                            wo_sb[jt][:, ys * 512:(ys + 1) * 512],
                            start=(jt == 0), stop=(jt == 3))
                    yst = stage_pool.tile([128, 512], f32, name="yst",
                                          tag="stage")
                    nc.vector.tensor_copy(yst[:], ps[:])
                    nc.sync.dma_start(
                        y_out[tt * 128:(tt + 1) * 128, ys * 512:(ys + 1) * 512],
                        yst[:])

            # emission order: pipeline proj stripes with attention waves
            def emit_body():
                qs0 = proj_stripe(0)
                qs1 = proj_stripe(1)
                for hp in range(4):
                    attn(hp, 0, qs0)
                for i in range(4):
                    nc.sync.dma_start(wo_sb[i][:],
                                      Wo[i * 128:(i + 1) * 128, :])
                qs2 = proj_stripe(2)
                for hp in range(4):
                    attn(hp, 1, qs1)
                for tt in range(4):
                    outproj(tt)
                qs3 = proj_stripe(3)
                for hp in range(4):
                    attn(hp, 2, qs2)
                for tt in range(4, 8):
                    outproj(tt)
                for hp in range(4):
                    attn(hp, 3, qs3)
                for tt in range(8, NT):
                    outproj(tt)

            if loop_k == 1:
                emit_body()
            else:
                with tc.For_i(0, loop_k, 1):
                    emit_body()

    nc.compile()
    return nc


_NC_CACHE = {}


def _get_program():
    if "nc" not in _NC_CACHE:
        _NC_CACHE["nc"] = _build_program()
    return _NC_CACHE["nc"]


def _make_tri():
    # valid (tk <= tq) in ST layout [tk, tq] = upper triangular incl diagonal
    p = np.arange(128)[:, None]
    f = np.arange(128)[None, :]
    return (p <= f).astype(ml_dtypes.bfloat16)


def make_in_maps(x, Wqkv, bqkv, Wout):
    tri = _make_tri()
    in_maps = []
    for c in range(NCORES):
        b, g = divmod(c, 2)
        j0 = g * JC
        Wq = Wqkv[:, j0:j0 + JC]
        Wk = Wqkv[:, C + j0:C + j0 + JC]
        Wv_ = Wqkv[:, 2 * C + j0:2 * C + j0 + JC]
        mmdt = np.float16 if PROJ_DT == "f16" else np.float32
        in_maps.append({
            "xT": np.ascontiguousarray(x[b].T).astype(mmdt),
            "Wqk": np.ascontiguousarray(
                np.concatenate([Wq, Wk], axis=1)).astype(mmdt),
            "Wv": np.ascontiguousarray(Wv_).astype(mmdt),
            "Wo": np.ascontiguousarray(Wout[j0:j0 + JC, :]).astype(mmdt),
            "bqk": np.ascontiguousarray(
                np.concatenate([bqkv[j0:j0 + JC], bqkv[C + j0:C + j0 + JC]])),
            "bv": np.ascontiguousarray(bqkv[2 * C + j0:2 * C + j0 + JC]),
            "tri": tri,
        })
    return in_maps


def kernel(x, Wqkv, bqkv, Wout, bout):
    from concourse.bass_utils import run_bass_kernel_spmd

    x = np.asarray(x, dtype=np.float32)
    Wqkv = np.asarray(Wqkv, dtype=np.float32)
    bqkv = np.asarray(bqkv, dtype=np.float32)
    Wout = np.asarray(Wout, dtype=np.float32)
    bout = np.asarray(bout, dtype=np.float32)

    in_maps = make_in_maps(x, Wqkv, bqkv, Wout)
    nc = _get_program()
    trace = bool(int(os.environ.get("KERNEL_TRACE", "0")))
    res = run_bass_kernel_spmd(nc, in_maps, list(range(NCORES)), trace=trace)
    results = res.results
    _NC_CACHE["last_exec_time_ns"] = res.exec_time_ns

    out = np.zeros((B, T, C), dtype=np.float32)
    k = np.zeros((B, H, T, HD), dtype=np.float32)
    v = np.zeros((B, H, T, HD), dtype=np.float32)
    for c in range(NCORES):
        b, g = divmod(c, 2)
        r = results[c]
        out[b] += r["y_out"]
        for lh in range(HPC):
            h = g * HPC + lh
            k[b, h] = r["kT_out"][lh * HD:(lh + 1) * HD, :].T
            v[b, h] = r["v_out"][:, lh * HD:(lh + 1) * HD]
    out += bout[None, None, :]
    return out, k, v
